# revision 1
# baseline (speedup 1.0000x reference)
"""CEM sampling kernel for Trainium2, 8-core SPMD (population sharded).

Pipeline per core (512 of 4096 population members):
  1. DTW min-plus DP over [128x128] cost tables via tensor_tensor_scan:
     all 4 population tiles packed into one [128, 516] row buffer with
     +inf separators, so each DP row is 2 DVE ops (shifted min + scan).
  2. AllGather local dists -> global [4096]; rank-count against own
     dists to get the global top-K elite mask without sorting.
  3. Weighted mean / E[x^2] partial sums over own noise shard
     (actions computed in place on ACT+GPSIMD during the DTW window),
     AllReduce partials, finish tiny [128,32] math, write [2,T,1,A].
"""

import os
import sys

for _p in ("/opt/trn_rl_repo", "/root/.axon_site/_ro/trn_rl_repo"):
    if _p not in sys.path:
        sys.path.insert(0, _p)

import numpy as np

import concourse.bass as bass
import concourse.bacc as bacc
import concourse.tile as tile
from concourse import mybir
from concourse import bass_utils

F32 = mybir.dt.float32
ALU = mybir.AluOpType
ACTF = mybir.ActivationFunctionType

P, T, A = 4096, 128, 32
NCORES = 8
PL = P // NCORES          # 512 population per core
NT = PL // 128            # 4 tiles of 128 on the partition dim
S = T + 1                 # 129: segment stride (128 cols + separator)
W = NT * S                # 516: packed row width
K = int(P * 0.1)          # 409
TEMP, MOM, MIN_STD = 0.5, 0.1, 0.05
INF = 1.0e30
RCH = int(os.environ.get("CEM_RCH", "8"))  # DP rows per streamed cost chunk
NCHUNK = T // RCH
GROUP = [list(range(NCORES))]

_CACHE = {}


def _build(stage=9, single=False):
    nc = bacc.Bacc(
        "TRN2",
        target_bir_lowering=False,
        debug=False,
        num_devices=1 if single else NCORES,
    )
    obs_d = nc.dram_tensor("obs", [PL, T, T], F32, kind="ExternalInput")
    means_d = nc.dram_tensor("means", [T, 1, A], F32, kind="ExternalInput")
    stds_d = nc.dram_tensor("stds", [T, 1, A], F32, kind="ExternalInput")
    noise_d = nc.dram_tensor("noise", [T, PL, A], F32, kind="ExternalInput")
    out_d = nc.dram_tensor("out", [2, T, 1, A], F32, kind="ExternalOutput")

    with tile.TileContext(nc) as tc:
        with (
            tc.tile_pool(name="main", bufs=1) as mp,
            tc.tile_pool(name="cwin", bufs=int(os.environ.get("CEM_CBUFS", "3"))) as cp,
            tc.tile_pool(name="dram", bufs=1, space="DRAM") as dp,
        ):
            # ---- stats-stage tiles; DMA early so actions overlap DTW
            noise_t = mp.tile([T, PL, A], F32)
            means_t = mp.tile([T, A], F32)
            stds_t = mp.tile([T, A], F32)
            nc.sync.dma_start(means_t[:], means_d[:, 0, :])
            nc.sync.dma_start(stds_t[:], stds_d[:, 0, :])

            def _actions_block():
                # noise prefetch + actions = clip(means + stds * noise) in
                # place, per action dim. ACT does the affine, GPSIMD the
                # clip: both idle during DTW. Traced after the first cost
                # chunks so their DMAs win the queue race.
                nc.sync.dma_start(noise_t[:], noise_d[:, :, :])
                if stage < 1:
                    return
                for a in range(A):
                    sl = noise_t[:, :, a]
                    nc.scalar.activation(
                        sl,
                        sl,
                        ACTF.Identity,
                        bias=means_t[:, a : a + 1],
                        scale=stds_t[:, a : a + 1],
                    )
                    nc.gpsimd.tensor_scalar(
                        sl, sl, 1.0, -1.0, op0=ALU.min, op1=ALU.max
                    )

            # ---- DTW DP over packed rows
            pbuf = mp.tile([128, W + 1], F32)
            ubuf = mp.tile([128, W], F32)
            nc.vector.memset(pbuf[:], INF)
            for k in range(NT):
                nc.vector.memset(pbuf[:, k * S : k * S + 1], 0.0)

            chunk_rows = [RCH] * (T // RCH)
            assert sum(chunk_rows) == T
            r0 = 0
            for c, rows in enumerate(chunk_rows):
                cb = cp.tile([128, rows, NT, S], F32, tag="cw")
                for k in range(NT):
                    nc.sync.dma_start(
                        cb[:, :, k, 0:T],
                        obs_d[k * 128 : (k + 1) * 128, r0 : r0 + rows, :],
                    )
                nc.gpsimd.memset(cb[:, :, :, T:S], INF)
                if c == 2:
                    _actions_block()
                for r in range(rows):
                    crow = cb[:, r].rearrange("p k j -> p (k j)")
                    nc.vector.tensor_tensor(
                        ubuf[:], pbuf[:, 0:W], pbuf[:, 1 : W + 1], op=ALU.min
                    )
                    nc.vector.tensor_tensor_scan(
                        pbuf[:, 1 : W + 1],
                        ubuf[:],
                        crow,
                        INF,
                        op0=ALU.min,
                        op1=ALU.add,
                    )
                    if c == 0 and r == 0:
                        # tile 0's left-boundary slot is never rewritten by
                        # the scan; after row 0 it must be +inf (D[i,0]).
                        nc.vector.memset(pbuf[:, 0:1], INF)
                r0 += rows

            # own dists: last col of each packed segment -> [128, NT]
            down = mp.tile([128, NT], F32)
            for k in range(NT):
                nc.vector.tensor_copy(
                    down[:, k : k + 1], pbuf[:, k * S + T : k * S + T + 1]
                )

            if stage >= 2:
                # ---- AllGather dists (tiny)
                ld = dp.tile([PL], F32)
                gd = dp.tile([P], F32)
                nc.sync.dma_start(ld.rearrange("(k p) -> p k", p=128), down[:])
                if single:
                    for cc in range(NCORES):
                        nc.sync.dma_start(gd[cc * PL : (cc + 1) * PL], ld[:])
                else:
                    nc.gpsimd.collective_compute(
                        "AllGather",
                        ALU.bypass,
                        replica_groups=GROUP,
                        ins=[ld.opt()],
                        outs=[gd.opt()],
                    )

            if stage >= 3:
                # broadcast global dists across partitions: 0-stride DMA
                # re-reads the 16KB vector once per partition; two halves so
                # the rank compares overlap the second half's transfer
                PH = P // 2
                gdb = mp.tile([128, 2, PH], F32)
                for h in range(2):
                    _, gsrc = bass.broadcast_tensor_aps(
                        gdb[:, h],
                        gd[h * PH : (h + 1) * PH].rearrange("(o f) -> o f", o=1),
                    )
                    nc.sync.dma_start(gdb[:, h], gsrc)

                ming2 = mp.tile([128, 2], F32)
                ming = mp.tile([128, 1], F32)
                # rank of own dists = #(d_j < d_p) per half; elite iff sum < K
                rank8 = mp.tile([128, 2, NT], F32)
                rank4 = mp.tile([128, NT], F32)
                scratch = cp.tile([128, PH], F32, tag="cw")
                for h in range(2):
                    for k in range(NT):
                        nc.vector.tensor_scalar(
                            scratch[:],
                            gdb[:, h],
                            down[:, k : k + 1],
                            None,
                            op0=ALU.is_lt,
                            op1=ALU.add,
                            accum_out=rank8[:, h, k : k + 1],
                        )
                    nc.vector.tensor_reduce(
                        ming2[:, h : h + 1],
                        gdb[:, h],
                        axis=mybir.AxisListType.X,
                        op=ALU.min,
                    )
                nc.vector.tensor_tensor(
                    rank4[:], rank8[:, 0], rank8[:, 1], op=ALU.add
                )
                nc.vector.tensor_reduce(
                    ming[:], ming2[:], axis=mybir.AxisListType.X, op=ALU.min
                )
                mask4 = mp.tile([128, NT], F32)
                nc.vector.tensor_scalar(
                    mask4[:], rank4[:], float(K), None, op0=ALU.is_lt
                )

                # w = mask * exp(TEMP*(min - d))
                biast = mp.tile([128, 1], F32)
                nc.vector.tensor_scalar(biast[:], ming[:], TEMP, None, op0=ALU.mult)
                e4 = mp.tile([128, NT], F32)
                nc.scalar.activation(
                    e4[:], down[:], ACTF.Exp, bias=biast[:, 0:1], scale=-TEMP
                )
                w4 = mp.tile([128, NT], F32)
                nc.vector.tensor_tensor(w4[:], e4[:], mask4[:], op=ALU.mult)

                # broadcast own weights along partitions: [128, PL]
                wl = dp.tile([PL], F32)
                nc.sync.dma_start(wl.rearrange("(k p) -> p k", p=128), w4[:])
                wrow = mp.tile([128, PL], F32)
                _, wsrc = bass.broadcast_tensor_aps(
                    wrow[:], wl.rearrange("(o f) -> o f", o=1)
                )
                nc.sync.dma_start(wrow[:], wsrc)

                slocal = mp.tile([128, 1], F32)
                nc.vector.tensor_reduce(
                    slocal[:], wrow[:], axis=mybir.AxisListType.X, op=ALU.add
                )

            if stage >= 4:
                # ---- weighted partial sums over own shard, in a-halves:
                # wa = w*act (broadcast w along a), act^2*w in place over
                # noise, then one strided reduce over p per quantity.
                num1 = mp.tile([128, A], F32)
                num2 = mp.tile([128, A], F32)
                AH = A // 2
                wah = cp.tile([128, PL, AH], F32, tag="wah", bufs=1)
                waa_dump = mp.tile([128, PL], F32)
                wrow3 = wrow[:].rearrange("t (p o) -> t p o", o=1)
                DSPL = 8  # a-columns per half on DVE; rest on GPSIMD
                for h in range(2):
                    a0 = h * AH
                    for eng, lo, hi in (
                        (nc.vector, 0, DSPL),
                        (nc.gpsimd, DSPL, AH),
                    ):
                        na = noise_t[:, :, a0 + lo : a0 + hi]
                        wv = wah[:, :, lo:hi]
                        b0, b1 = bass.broadcast_tensor_aps(na, wrow3)
                        eng.tensor_tensor(wv, b0, b1, op=ALU.mult)
                        eng.tensor_tensor(na, wv, na, op=ALU.mult)
                    nc.vector.tensor_reduce(
                        num1[:, a0 : a0 + AH],
                        wah[:].rearrange("t p a -> t a p"),
                        axis=mybir.AxisListType.X,
                        op=ALU.add,
                    )
                    for a in range(a0, a0 + AH):
                        nc.scalar.activation(
                            waa_dump[:],
                            noise_t[:, :, a],
                            ACTF.Identity,
                            accum_out=num2[:, a : a + 1],
                        )

            if stage >= 5:
                # ---- AllReduce partials: [num1 | num2 | S]
                NTOT = 2 * T * A + T
                arin = dp.tile([NTOT], F32)
                arout = dp.tile([NTOT], F32)
                nc.sync.dma_start(
                    arin[0 : T * A].rearrange("(p a) -> p a", a=A), num1[:]
                )
                nc.sync.dma_start(
                    arin[T * A : 2 * T * A].rearrange("(p a) -> p a", a=A), num2[:]
                )
                nc.sync.dma_start(
                    arin[2 * T * A : NTOT].rearrange("(p a) -> p a", a=1), slocal[:]
                )
                if single:
                    nc.sync.dma_start(arout[:], arin[:])
                else:
                    nc.gpsimd.collective_compute(
                        "AllReduce",
                        ALU.add,
                        replica_groups=GROUP,
                        ins=[arin.opt()],
                        outs=[arout.opt()],
                    )
                rn12 = mp.tile([128, 2, A], F32)
                rs = mp.tile([128, 1], F32)
                nc.sync.dma_start(
                    rn12[:],
                    arout[0 : 2 * T * A].rearrange("(q p a) -> p q a", q=2, a=A),
                )
                rn1 = rn12[:, 0]
                rn2 = rn12[:, 1]
                nc.sync.dma_start(
                    rs[:], arout[2 * T * A : NTOT].rearrange("(p a) -> p a", a=1)
                )

                # ---- final statistics
                inv = mp.tile([128, 1], F32)
                nc.vector.reciprocal(inv[:], rs[:])
                mh = mp.tile([128, A], F32)
                nc.vector.tensor_scalar(
                    mh[:], rn1, inv[:, 0:1], None, op0=ALU.mult
                )
                q = mp.tile([128, A], F32)
                nc.vector.tensor_scalar(
                    q[:], rn2, inv[:, 0:1], None, op0=ALU.mult
                )
                msq = mp.tile([128, A], F32)
                nc.vector.tensor_tensor(msq[:], mh[:], mh[:], op=ALU.mult)
                var = mp.tile([128, A], F32)
                nc.vector.tensor_tensor(var[:], q[:], msq[:], op=ALU.subtract)
                nc.vector.tensor_scalar(var[:], var[:], 0.0, None, op0=ALU.max)
                stdv = mp.tile([128, A], F32)
                nc.scalar.sqrt(stdv[:], var[:])
                nc.vector.tensor_scalar(
                    stdv[:], stdv[:], MIN_STD, 1.0, op0=ALU.max, op1=ALU.min
                )
                mnew = mp.tile([128, A], F32)
                nc.vector.tensor_scalar(
                    mh[:], mh[:], 1.0 - MOM, None, op0=ALU.mult
                )
                nc.vector.scalar_tensor_tensor(
                    mnew[:], means_t[:], MOM, mh[:], op0=ALU.mult, op1=ALU.add
                )
                nc.sync.dma_start(out_d[0, :, 0, :], mnew[:])
                nc.sync.dma_start(out_d[1, :, 0, :], stdv[:])
            else:
                # bisect debug output
                dbg = mp.tile([128, A], F32)
                nc.vector.memset(dbg[:], 0.0)
                if stage >= 3:
                    nc.vector.tensor_copy(dbg[:, 0:NT], w4[:])
                    nc.vector.tensor_copy(dbg[:, NT : NT + 1], slocal[:])
                elif stage >= 0:
                    nc.vector.tensor_copy(dbg[:, 0:NT], down[:])
                if stage == 2:
                    gdbg = mp.tile([128, A], F32)
                    nc.sync.dma_start(
                        gdbg[:],
                        gd[0 : 128 * A].rearrange("(p a) -> p a", a=A),
                    )
                    nc.vector.tensor_copy(dbg[:, 4:8], gdbg[:, 0:4])
                nc.sync.dma_start(out_d[0, :, 0, :], dbg[:])
                nc.sync.dma_start(out_d[1, :, 0, :], dbg[:])

    nc.compile()
    return nc


def _get_nc(stage=None, single=None):
    # staged/single variants exist only for the dev harness (test.py);
    # kernel() always runs the full 8-core program.
    if stage is None:
        stage = int(os.environ.get("CEM_STAGE", "9"))
    if single is None:
        single = bool(int(os.environ.get("CEM_SINGLE", "0")))
    key = ("nc", stage, single)
    if key not in _CACHE:
        _CACHE[key] = _build(stage, single)
    return _CACHE[key]


def kernel(**inputs):
    obs = np.ascontiguousarray(np.asarray(inputs["obs_diffs"], np.float32))
    means = np.ascontiguousarray(np.asarray(inputs["means"], np.float32))
    stds = np.ascontiguousarray(np.asarray(inputs["stds"], np.float32))
    noise = np.ascontiguousarray(np.asarray(inputs["noise"], np.float32))

    nc = _get_nc(stage=9, single=False)
    in_maps = []
    for c in range(NCORES):
        in_maps.append(
            {
                "obs": obs[c * PL : (c + 1) * PL],
                "means": means,
                "stds": stds,
                "noise": np.ascontiguousarray(noise[:, c * PL : (c + 1) * PL, :]),
            }
        )
    res = bass_utils.run_bass_kernel_spmd(
        nc, in_maps, core_ids=list(range(NCORES))
    )
    out = np.asarray(res.results[0]["out"], np.float32)
    return out.reshape(2, T, 1, A)



# revision 20
# speedup vs baseline: 1.4589x; 1.4589x over previous
"""CEM sampling kernel for Trainium2, 8-core SPMD (population sharded).

Per core (512 of 4096 members), one fused program:

  Window (overlapped with the 42MB obs+noise HBM stream, ~117us):
   - DTW min-plus DP entirely on DVE (the scan/min ops exist only
     there): two packed pair-chains [t0|sep|t1] and [t2|sep|t3], DP
     state in fp16 (2x-mode mins; the scan's carry is internally fp32
     and the f32 cost rows are never rounded, so only the stored row
     values quantize).  ~1.1us/row.
   - Actions: ACT computes bf16 act = means + stds*noise per action
     dim, Pool clips in bf16, PE transposes [t,p] blocks to a
     population-major bf16 layout, ACT copies PSUM->SBUF and squares.
  Tail (~35us): AllGather dists; top-K via the gpsimd kth_largest
     library op on the [128,32] negated global dists (exact K-th
     threshold, replaces rank compares and broadcasts); weights; the
     weighted mean / E[x^2] reductions as 64 bf16 PE matmuls (with a
     p-state warmup) accumulating in PSUM; AllReduce; closing stats.
"""

import os
import sys

for _p in ("/opt/trn_rl_repo", "/root/.axon_site/_ro/trn_rl_repo"):
    if _p not in sys.path:
        sys.path.insert(0, _p)

import numpy as np

import concourse.bass as bass
import concourse.bacc as bacc
import concourse.bass_isa as bass_isa
import concourse.tile as tile
from concourse import mybir
from concourse import bass_utils
from concourse.masks import make_identity

F32 = mybir.dt.float32
FP16 = mybir.dt.float16
BF16 = mybir.dt.bfloat16
ALU = mybir.AluOpType
ACTF = mybir.ActivationFunctionType

P, T, A = 4096, 128, 32
NCORES = 8
PL = P // NCORES          # 512 population per core
NT = PL // 128            # 4 tiles of 128 on the partition dim
K = int(P * 0.1)          # 409
TEMP, MOM, MIN_STD = 0.5, 0.1, 0.05
INFDP = 30000.0           # fp16-safe stand-in for +inf in the DP
RCH = int(os.environ.get("CEM_RCH", "8"))   # DP rows per streamed chunk
NCHUNK = T // RCH
CBUFS = int(os.environ.get("CEM_CBUFS", "3"))
WARM = int(os.environ.get("CEM_WARM", "10"))  # PE p-state warmup matmuls
DPDT = FP16 if os.environ.get("CEM_DPDT", "fp16") == "fp16" else F32
GROUP = [list(range(NCORES))]

# packed cost-row layout: [t0(128) sep t1(128) | t2(128) sep t3(128)]
CW = 257                  # cost width of one pair-chain
CWF = 514
SEP1, SEP2 = 128, 385
DMAP = {0: 0, 1: 129, 2: 257, 3: 386}  # pop tile -> flat cost column

_CACHE = {}


def _build(stage=9, single=False):
    nc = bacc.Bacc(
        "TRN2",
        target_bir_lowering=False,
        debug=False,
        num_devices=1 if single else NCORES,
    )
    obs_d = nc.dram_tensor("obs", [PL, T, T], F32, kind="ExternalInput")
    means_d = nc.dram_tensor("means", [T, 1, A], F32, kind="ExternalInput")
    stds_d = nc.dram_tensor("stds", [T, 1, A], F32, kind="ExternalInput")
    noise_d = nc.dram_tensor("noise", [T, PL, A], F32, kind="ExternalInput")
    out_d = nc.dram_tensor("out", [2, T, 1, A], F32, kind="ExternalOutput")

    with tile.TileContext(nc) as tc:
        with (
            tc.tile_pool(name="main", bufs=1) as mp,
            tc.tile_pool(name="dram", bufs=1, space="DRAM") as dp,
        ):
            # ---- small persistent tiles
            means_t = mp.tile([T, A], F32)
            stds_t = mp.tile([T, A], F32)
            nc.sync.dma_start(means_t[:], means_d[:, 0, :])
            nc.sync.dma_start(stds_t[:], stds_d[:, 0, :])
            ident = mp.tile([128, 128], BF16)
            make_identity(nc, ident[:])

            # actions (bf16), noise staging quarters, transposed layouts
            actb = mp.tile([T, PL, A], BF16)
            utile = mp.tile([128, 2 * PL * A // 4], F32)  # [128, 8192]
            nhq = [
                utile[:, 0:4096].rearrange("t (p a) -> t p a", a=A),
                utile[:, 4096:8192].rearrange("t (p a) -> t p a", a=A),
            ]
            actT = mp.tile([128, NT, T, A], BF16)
            act2T = mp.tile([128, NT, T, A], BF16)

            # ---- DTW state: two packed pair-chains, ping-pong, DPDT
            h01a = mp.tile([128, CW + 1], DPDT)
            h01b = mp.tile([128, CW + 1], DPDT)
            h23a = mp.tile([128, CW + 1], DPDT)
            h23b = mp.tile([128, CW + 1], DPDT)
            ub01 = mp.tile([128, CW], DPDT)
            ub23 = mp.tile([128, CW], DPDT)
            for t_ in (h01a, h01b, h23a, h23b):
                nc.vector.memset(t_[:], INFDP)
            # D[0][0] = 0 for each tile (pair cols 0 and 129)
            nc.vector.memset(h01a[:, 0:1], 0.0)
            nc.vector.memset(h01a[:, 129:130], 0.0)
            nc.vector.memset(h23a[:, 0:1], 0.0)
            nc.vector.memset(h23a[:, 129:130], 0.0)

            down = mp.tile([128, NT], F32)
            ch01 = (h01a, h01b)
            ch23 = (h23a, h23b)

            def dtw_row(i, cb, r):
                crow = cb[:, r]
                A1, B1 = ch01[i % 2], ch01[(i + 1) % 2]
                A2, B2 = ch23[i % 2], ch23[(i + 1) % 2]
                nc.vector.tensor_tensor(
                    ub01[:], A1[:, 0:CW], A1[:, 1 : CW + 1], op=ALU.min
                )
                nc.vector.tensor_tensor(
                    ub23[:], A2[:, 0:CW], A2[:, 1 : CW + 1], op=ALU.min
                )
                nc.vector.tensor_tensor_scan(
                    B1[:, 1 : CW + 1], ub01[:], crow[:, 0:CW], INFDP,
                    op0=ALU.min, op1=ALU.add,
                )
                nc.vector.tensor_tensor_scan(
                    B2[:, 1 : CW + 1], ub23[:], crow[:, CW:CWF], INFDP,
                    op0=ALU.min, op1=ALU.add,
                )
                if i == 0:
                    # D[i>0][0] = INF at the never-rewritten left columns
                    nc.vector.memset(h01a[:, 0:1], INFDP)
                    nc.vector.memset(h23a[:, 0:1], INFDP)

            # ---- actions pipeline pieces (emitted interleaved with DTW)
            def noise_dma(q):
                nc.sync.dma_start(
                    nhq[q % 2][:], noise_d[:, q * 128 : (q + 1) * 128, :]
                )

            def affine(q):
                for a in range(A):
                    nc.scalar.activation(
                        actb[:, q * 128 : (q + 1) * 128, a],
                        nhq[q % 2][:, :, a],
                        ACTF.Identity,
                        bias=means_t[:, a : a + 1],
                        scale=stds_t[:, a : a + 1],
                    )

            def clip(k):
                v = actb[:, k * 128 : (k + 1) * 128, :].rearrange(
                    "t p a -> t (p a)"
                )
                nc.gpsimd.tensor_scalar(
                    v, v, 1.0, -1.0, op0=ALU.min, op1=ALU.max
                )

            def transposes(tpp, k):
                for a in range(A):
                    pt = tpp.tile([128, 128], BF16, tag="tp")
                    nc.tensor.transpose(
                        pt[:],
                        actb[:, k * 128 : (k + 1) * 128, a],
                        ident[:],
                    )
                    nc.scalar.activation(
                        actT[:, k, :, a], pt[:], ACTF.Copy
                    )

            def square(k):
                nc.scalar.activation(
                    act2T[:, k].rearrange("t a b -> t (a b)"),
                    actT[:, k].rearrange("t a b -> t (a b)"),
                    ACTF.Square,
                )

            # ---- window: obs chunks + DTW rows + action stages
            with tc.tile_pool(name="cwin", bufs=CBUFS) as cp, \
                 tc.tile_pool(name="psum_tp", bufs=4, space="PSUM") as tpp:
                cbs = []

                def chunk_dma(c):
                    cb = cp.tile([128, RCH, CWF], F32, tag="cw")
                    for k in range(NT):
                        o = DMAP[k]
                        nc.sync.dma_start(
                            cb[:, :, o : o + T],
                            obs_d[k * 128 : (k + 1) * 128,
                                  c * RCH : (c + 1) * RCH, :],
                        )
                    # refresh both INF separators each generation
                    nc.scalar.activation(
                        cb[:, :, SEP1 :: CW].rearrange("t r s -> t (r s)"),
                        means_t[:, 0 : 2 * RCH],
                        ACTF.Copy, bias=INFDP, scale=0.0,
                    )
                    return cb

                cbs.append(chunk_dma(0))
                noise_dma(0)
                for c in range(1, min(CBUFS, NCHUNK)):
                    cbs.append(chunk_dma(c))

                acts = {
                    0: [lambda: affine(0), lambda: noise_dma(1)],
                    1: [lambda: affine(1), lambda: noise_dma(2)],
                    2: [lambda: clip(0)],
                    3: [lambda: affine(2), lambda: noise_dma(3),
                        lambda: clip(1)],
                    4: [lambda: transposes(tpp, 0)],
                    5: [lambda: affine(3), lambda: clip(2)],
                    6: [lambda: transposes(tpp, 1), lambda: square(0)],
                    7: [lambda: clip(3)],
                    8: [lambda: transposes(tpp, 2), lambda: square(1)],
                    9: [lambda: transposes(tpp, 3)],
                    10: [lambda: square(2)],
                    11: [lambda: square(3)],
                }

                for c in range(NCHUNK):
                    cb = cbs[c]
                    for r in range(RCH):
                        dtw_row(c * RCH + r, cb, r)
                    if c + CBUFS < NCHUNK:
                        cbs.append(chunk_dma(c + CBUFS))
                    if stage >= 1:
                        for th in acts.get(c, []):
                            th()

            # own dists from the final (even-side) buffers, fp16 -> f32
            nc.scalar.activation(down[:, 0:1], h01a[:, 128:129], ACTF.Copy)
            nc.scalar.activation(down[:, 1:2], h01a[:, 257:258], ACTF.Copy)
            nc.scalar.activation(down[:, 2:3], h23a[:, 128:129], ACTF.Copy)
            nc.scalar.activation(down[:, 3:4], h23a[:, 257:258], ACTF.Copy)

            if stage >= 2:
                # ---- AllGather dists (tiny)
                ld = dp.tile([PL], F32)
                gd = dp.tile([P], F32)
                nc.sync.dma_start(ld.rearrange("(k p) -> p k", p=128), down[:])
                if single:
                    _, lsrc = bass.broadcast_tensor_aps(
                        gd.rearrange("(r f) -> r f", r=NCORES),
                        ld.rearrange("(o f) -> o f", o=1),
                    )
                    nc.sync.dma_start(
                        gd.rearrange("(r f) -> r f", r=NCORES), lsrc
                    )
                else:
                    nc.gpsimd.collective_compute(
                        "AllGather",
                        ALU.bypass,
                        replica_groups=GROUP,
                        ins=[ld.opt()],
                        outs=[gd.opt()],
                    )

            if stage >= 3:
                # ---- top-K threshold via gpsimd kth_largest on -dists
                gdsq = mp.tile([128, P // 128], F32)
                nc.sync.dma_start(
                    gdsq[:], gd.rearrange("(p f) -> p f", p=128)
                )
                ngd = mp.tile([128, P // 128], F32)
                nc.vector.tensor_scalar(
                    ngd[:], gdsq[:], -1.0, None, op0=ALU.mult
                )
                kth = mp.tile([128, 2], F32)
                nc.gpsimd.kth_largest(
                    kth[:], ngd[:], P // 128, K + 3,
                    quantile=1.0 - (K - 0.5) / (P - 1),
                )
                # kth col1 = desc[k_adj+1] = -s[K] ; mask = d < s[K]
                thb = mp.tile([128, 2], F32)
                nc.gpsimd.partition_broadcast(thb[:], kth[0:1, :])
                thneg = mp.tile([128, 1], F32)
                nc.vector.tensor_scalar(
                    thneg[:], thb[:, 1:2], -1.0, None, op0=ALU.mult
                )
                # softmax shift: any global constant cancels exactly; use
                # gd[0] (safe unless the dist spread nears 176/TEMP).
                dref = mp.tile([128, 1], F32)
                nc.gpsimd.partition_broadcast(dref[:], gdsq[0:1, 0:1])
                biast = mp.tile([128, 1], F32)
                nc.gpsimd.tensor_scalar(
                    biast[:], dref[:], TEMP, None, op0=ALU.mult
                )
                mask4 = mp.tile([128, NT], F32)
                nc.vector.tensor_scalar(
                    mask4[:], down[:], thneg[:, 0:1], None, op0=ALU.is_lt
                )
                e4 = mp.tile([128, NT], F32)
                nc.scalar.activation(
                    e4[:], down[:], ACTF.Exp, bias=biast[:, 0:1], scale=-TEMP
                )
                w4 = mp.tile([128, NT], F32)
                nc.vector.tensor_tensor(w4[:], e4[:], mask4[:], op=ALU.mult)
                wb = mp.tile([128, NT], BF16)
                nc.scalar.activation(wb[:], w4[:], ACTF.Copy)
                # sum of weights across members (free then partitions)
                slocal = mp.tile([128, 1], F32)
                nc.vector.tensor_reduce(
                    slocal[:], w4[:], axis=mybir.AxisListType.X, op=ALU.add
                )
                swr = mp.tile([128, 1], F32)
                nc.gpsimd.partition_all_reduce(
                    swr[:], slocal[:], 128, bass_isa.ReduceOp.add
                )
                # dnb: bf16 dists, ready at DTW end -- gates the PE warmup
                dnb = mp.tile([128, NT], BF16)
                nc.scalar.activation(dnb[:], down[:], ACTF.Copy)

            if stage >= 4:
                # ---- weighted sums as PE matmuls accumulating over tiles
                _pst_cm = tc.tile_pool(name="psum_st", bufs=1, space="PSUM")
                pst = _pst_cm.__enter__()
                sts = []
                for c in range(8):
                    st = pst.tile([128, 512], F32, tag=f"st{c}")
                    sts.append(st)
                # PE p-state warmup: junk matmuls gated on the dists; their
                # outputs are reset by the first start=True real matmul.
                for wi in range(WARM):
                    nc.tensor.matmul(
                        sts[wi % 8][0:1, :],
                        dnb[:, 0:1],
                        actT[:, wi % NT, (wi % 8) * 16 : (wi % 8) * 16 + 16, :],
                        start=True, stop=True, skip_group_check=True,
                    )
                # staging rows alias dead actb (32-aligned partitions)
                arsc = actb[:].rearrange("t p a -> t (p a)").bitcast(F32)
                arsb_m = arsc[0:1, 0 : T * A]
                arsb_s = arsc[32:33, 0 : T * A]
                for c in range(8):
                    for k in range(NT):
                        nc.tensor.matmul(
                            sts[c][0:1, :],
                            wb[:, k : k + 1],
                            actT[:, k, c * 16 : (c + 1) * 16, :],
                            start=(k == 0), stop=(k == NT - 1),
                        )
                    nc.scalar.activation(
                        arsb_m[:, c * 512 : (c + 1) * 512],
                        sts[c][0:1, :], ACTF.Copy,
                    )
                for c in range(8):
                    for k in range(NT):
                        nc.tensor.matmul(
                            sts[c][32:33, :],
                            wb[:, k : k + 1],
                            act2T[:, k, c * 16 : (c + 1) * 16, :],
                            start=(k == 0), stop=(k == NT - 1),
                        )
                    nc.vector.tensor_copy(
                        arsb_s[:, c * 512 : (c + 1) * 512],
                        sts[c][32:33, :],
                    )

                NTOT = 2 * T * A + 1
                arin = dp.tile([NTOT], F32)
                arout = dp.tile([NTOT], F32)
                nc.sync.dma_start(
                    arin[0 : T * A].rearrange("(o f) -> o f", o=1), arsb_m[:]
                )
                nc.sync.dma_start(
                    arin[T * A : 2 * T * A].rearrange("(o f) -> o f", o=1),
                    arsb_s[:],
                )
                nc.sync.dma_start(
                    arin[2 * T * A : NTOT].rearrange("(o f) -> o f", o=1),
                    swr[0:1, 0:1],
                )
                if single:
                    nc.sync.dma_start(arout[:], arin[:])
                else:
                    nc.gpsimd.collective_compute(
                        "AllReduce",
                        ALU.add,
                        replica_groups=GROUP,
                        ins=[arin.opt()],
                        outs=[arout.opt()],
                    )
                _pst_cm.__exit__(None, None, None)

            if stage >= 5:
                # ---- final statistics
                rn12 = mp.tile([128, 2, A], F32)
                nc.sync.dma_start(
                    rn12[:],
                    arout[0 : 2 * T * A].rearrange(
                        "(q t a) -> t q a", q=2, t=T
                    ),
                )
                rs = mp.tile([128, 1], F32)
                _, rssrc = bass.broadcast_tensor_aps(
                    rs[:],
                    arout[2 * T * A : NTOT].rearrange("(o f) -> o f", o=1),
                )
                nc.sync.dma_start(rs[:], rssrc)
                rn1 = rn12[:, 0]
                rn2 = rn12[:, 1]
                inv = mp.tile([128, 1], F32)
                nc.vector.reciprocal(inv[:], rs[:])
                mh = mp.tile([128, A], F32)
                nc.vector.tensor_scalar(
                    mh[:], rn1, inv[:, 0:1], None, op0=ALU.mult
                )
                q = mp.tile([128, A], F32)
                nc.vector.tensor_scalar(
                    q[:], rn2, inv[:, 0:1], None, op0=ALU.mult
                )
                msq = mp.tile([128, A], F32)
                nc.vector.tensor_tensor(msq[:], mh[:], mh[:], op=ALU.mult)
                var = mp.tile([128, A], F32)
                nc.vector.tensor_tensor(var[:], q[:], msq[:], op=ALU.subtract)
                nc.vector.tensor_scalar(var[:], var[:], 0.0, None, op0=ALU.max)
                stdv = mp.tile([128, A], F32)
                nc.scalar.sqrt(stdv[:], var[:])
                nc.vector.tensor_scalar(
                    stdv[:], stdv[:], MIN_STD, 1.0, op0=ALU.max, op1=ALU.min
                )
                mnew = mp.tile([128, A], F32)
                nc.vector.tensor_scalar(
                    mh[:], mh[:], 1.0 - MOM, None, op0=ALU.mult
                )
                nc.vector.scalar_tensor_tensor(
                    mnew[:], means_t[:], MOM, mh[:], op0=ALU.mult, op1=ALU.add
                )
                nc.sync.dma_start(out_d[0, :, 0, :], mnew[:])
                nc.sync.dma_start(out_d[1, :, 0, :], stdv[:])
            else:
                # bisect debug output
                dbg = mp.tile([128, A], F32)
                nc.vector.memset(dbg[:], 0.0)
                nc.vector.tensor_copy(dbg[:, 0:NT], down[:])
                if stage >= 3:
                    nc.vector.tensor_copy(dbg[:, 4 : 4 + NT], w4[:])
                    nc.vector.tensor_copy(dbg[:, 8:9], thneg[:])
                    nc.vector.tensor_copy(dbg[:, 9:10], swr[:])
                if stage == 2:
                    gdbg = mp.tile([128, A], F32)
                    nc.sync.dma_start(
                        gdbg[:],
                        gd[0 : 128 * A].rearrange("(p a) -> p a", a=A),
                    )
                    nc.vector.tensor_copy(dbg[:, 4:8], gdbg[:, 0:4])
                nc.sync.dma_start(out_d[0, :, 0, :], dbg[:])
                nc.sync.dma_start(out_d[1, :, 0, :], dbg[:])

    nc.compile()
    return nc


def _get_nc(stage=None, single=None):
    if stage is None:
        stage = int(os.environ.get("CEM_STAGE", "9"))
    if single is None:
        single = bool(int(os.environ.get("CEM_SINGLE", "0")))
    key = ("nc", stage, single)
    if key not in _CACHE:
        _CACHE[key] = _build(stage, single)
    return _CACHE[key]


def kernel(**inputs):
    obs = np.ascontiguousarray(np.asarray(inputs["obs_diffs"], np.float32))
    means = np.ascontiguousarray(np.asarray(inputs["means"], np.float32))
    stds = np.ascontiguousarray(np.asarray(inputs["stds"], np.float32))
    noise = np.ascontiguousarray(np.asarray(inputs["noise"], np.float32))

    nc = _get_nc(stage=9, single=False)
    in_maps = []
    for c in range(NCORES):
        in_maps.append(
            {
                "obs": obs[c * PL : (c + 1) * PL],
                "means": means,
                "stds": stds,
                "noise": np.ascontiguousarray(noise[:, c * PL : (c + 1) * PL, :]),
            }
        )
    res = bass_utils.run_bass_kernel_spmd(
        nc, in_maps, core_ids=list(range(NCORES))
    )
    out = np.asarray(res.results[0]["out"], np.float32)
    return out.reshape(2, T, 1, A)
